# revision 22
# baseline (speedup 1.0000x reference)
"""Trainium2 Bass kernel for nn_CGLayer (gnn_message_passing).

Strategy (x-sharded over 8 cores, 32 target-rows each):
  Host: tiny phases — vmp = adj@v, first CG product + wn mixing -> mixed[L][x,M,d];
        fold CG tables x mixed into per-x stationary matrices Gt[x] [120k, 81p];
        rearrange wr -> wrT[x] [120, (y,d)=2048] so the big c'-contraction is a
        layout-perfect PE matmul; replicate sph -> strep[x] [81, 256].
  Device per core, per x: 1MB DMA of wrT[x];
        PE mm2: V[81,(y,d)] = Gt[x].T @ wrT[x]          (k=120, f=512 x4)
        DVE:    W = V * strep[x] (broadcast over d)
        PE mm3: acc[9,(y,d)] += SEL.T @ W                (k=81, accumulate over x)
  Host: sum partials over cores, per-l global-sum normalize, concat.
"""

import math
import sys
from contextlib import ExitStack

import numpy as np

sys.path.insert(0, "/opt/trn_rl_repo")

import concourse.bass as bass
import concourse.mybir as mybir
from concourse import tile
from concourse.tile import add_dep_helper
from concourse.bass_utils import run_bass_kernel_spmd

MAXL = 2
N = 256
TAU = 8
NCORES = 8
XC = N // NCORES  # 32 x-rows per core

# ---------------- CG coefficient tables (same math as reference) ----------------


def _cg_coeff(j1, m1, j2, m2, J, M):
    if m1 + m2 != M or J < abs(j1 - j2) or J > j1 + j2:
        return 0.0
    f = math.factorial
    pre = math.sqrt(
        (2 * J + 1) * f(J + j1 - j2) * f(J - j1 + j2) * f(j1 + j2 - J) / f(j1 + j2 + J + 1)
    )
    pre *= math.sqrt(f(J + M) * f(J - M) * f(j1 - m1) * f(j1 + m1) * f(j2 - m2) * f(j2 + m2))
    kmin = max(0, j2 - J - m1, j1 + m2 - J)
    kmax = min(j1 + j2 - J, j1 - m1, j2 + m2)
    s = 0.0
    for k in range(kmin, kmax + 1):
        s += (-1.0) ** k / (
            f(k) * f(j1 + j2 - J - k) * f(j1 - m1 - k) * f(j2 + m2 - k) * f(J - j2 + m1 + k) * f(J - j1 - m2 + k)
        )
    return pre * s


def _cg_tensor(l1, l2, L):
    C = np.zeros((2 * l1 + 1, 2 * l2 + 1, 2 * L + 1), np.float32)
    for m1 in range(-l1, l1 + 1):
        for m2 in range(-l2, l2 + 1):
            M = m1 + m2
            if -L <= M <= L:
                C[m1 + l1, m2 + l2, M + L] = _cg_coeff(l1, m1, l2, m2, L, M)
    return C


_CG = {}
for _l1 in range(MAXL + 1):
    for _l2 in range(MAXL + 1):
        for _L in range(abs(_l1 - _l2), min(_l1 + _l2, MAXL) + 1):
            _CG[(_l1, _l2, _L)] = _cg_tensor(_l1, _l2, _L)

# chunk (l1,l2) lists per L, in reference concat order
CHUNKS = {
    L: [(l1, l2) for l1 in range(MAXL + 1) for l2 in range(MAXL + 1) if abs(l1 - l2) <= L <= l1 + l2]
    for L in range(MAXL + 1)
}
KBASE = {0: 0, 1: 24, 2: 72}  # k-row base per L (widths 24/48/48) -> 120 total
KTOT = 120
PBASE = {0: 0, 1: 9, 2: 36}  # p-col base per L (widths 9*(2L+1)) -> 81 total
PTOT = 81
L2BASE = [0, 1, 4]  # m2' offset of l2-block within the 9 sph rows
QBASE = [0, 1, 4]  # out row base per L -> 9 total
QTOT = 9
FB = 4  # f-blocks of 512 over (y,d)=2048

# ---------------- device kernel ----------------

_NC = None
LAST_RESULTS = None


def _build_nc():
    f32 = mybir.dt.float32
    f32r = mybir.dt.float32r
    nc = bass.Bass(trn_type="TRN2")
    wrt = nc.dram_tensor("wrt", [XC, KTOT, N * TAU], f32r, kind="ExternalInput")
    gt = nc.dram_tensor("gt", [KTOT, XC, PTOT], f32r, kind="ExternalInput")
    strep = nc.dram_tensor("strep", [PTOT, XC, N], f32, kind="ExternalInput")
    sel = nc.dram_tensor("sel", [PTOT, QTOT], f32r, kind="ExternalInput")
    out = nc.dram_tensor("out", [QTOT, N * TAU], f32, kind="ExternalOutput")

    with ExitStack() as ctx:
        tc = ctx.enter_context(tile.TileContext(nc))
        const = ctx.enter_context(tc.tile_pool(name="const", bufs=1))
        wrt_pool = ctx.enter_context(tc.tile_pool(name="wrt", bufs=6))
        w_pool = ctx.enter_context(tc.tile_pool(name="w", bufs=4))
        v_psum = ctx.enter_context(tc.tile_pool(name="v", bufs=3, space="PSUM"))
        probe_psum = ctx.enter_context(tc.tile_pool(name="probe", bufs=1, space="PSUM"))
        acc_psum = ctx.enter_context(tc.tile_pool(name="acc", bufs=1, space="PSUM"))
        outp = ctx.enter_context(tc.tile_pool(name="outp", bufs=1))

        gt_t = const.tile([KTOT, XC, PTOT], f32r)
        nc.sync.dma_start(gt_t[:], gt[:])
        strep_t = const.tile([PTOT, XC, N], f32)
        nc.sync.dma_start(strep_t[:], strep[:])
        sel_t = const.tile([PTOT, QTOT], f32r)
        nc.sync.dma_start(sel_t[:], sel[:])

        accs = [
            acc_psum.tile([QTOT, 64, TAU], f32, name=f"acc{fb}", tag=f"acc{fb}")
            for fb in range(FB)
        ]
        probe = probe_psum.tile([1, 1], f32, name="probe", tag="probe")

        for x in range(XC):
            wt = wrt_pool.tile([KTOT, N * TAU], f32r)
            nc.sync.dma_start(wt[:], wrt[x])
            # tiny PE matmul touching wt so PE observes the DMA semaphore here;
            # the real matmuls then carry at most one wait (fp32r LDW limit).
            probe_mm = nc.tensor.matmul(
                probe[:], wt[0:1, 0:1].bitcast(f32), wt[0:1, 0:1].bitcast(f32), start=(x == 0), stop=(x == XC - 1)
            )
            for fb in range(FB):
                v = v_psum.tile([PTOT, 64, TAU], f32)
                mm2 = nc.tensor.matmul(
                    v[:],
                    gt_t[:, x, :],
                    wt[:, fb * 512 : (fb + 1) * 512],
                    start=True,
                    stop=True,
                )
                if fb == 0:
                    add_dep_helper(mm2.ins, probe_mm.ins, sync=True, reason="mm2 after probe so PE already observed wt DMA")
                w = w_pool.tile([PTOT, 64, TAU], f32r)
                nc.vector.tensor_tensor(
                    w[:],
                    v[:],
                    strep_t[:, x, fb * 64 : (fb + 1) * 64, None].to_broadcast((PTOT, 64, TAU)),
                    mybir.AluOpType.mult,
                )
                nc.tensor.matmul(
                    accs[fb][:],
                    sel_t[:],
                    w[:],
                    start=(x == 0),
                    stop=(x == XC - 1),
                )
        out_sb = outp.tile([QTOT, N * TAU], f32)
        for fb in range(FB):
            nc.vector.tensor_copy(out_sb[:, fb * 512 : (fb + 1) * 512], accs[fb][:])
        nc.sync.dma_start(out[:], out_sb[:])
    _split_multi_waits(nc)
    return nc


# walrus' compute-op structs hold a single sync-wait slot; hoist extra waits
# into standalone single-wait EventSemaphore ops on the same engine.
_NO_SPLIT_TYPES = {"InstEventSemaphore", "InstUnconditionalBranch", "InstCall", "InstISA"}


def _split_multi_waits(nc):
    for fn in nc.m.functions:
        for bb in fn.blocks:
            insts = list(bb.instructions)
            out_list = []
            changed = False
            for i in insts:
                si = i.sync_info
                if (
                    si is not None
                    and len(si.on_wait) > 1
                    and type(i).__name__ not in _NO_SPLIT_TYPES
                ):
                    for j, wcond in enumerate(list(si.on_wait[:-1])):
                        ev = mybir.InstEventSemaphore(
                            name=f"wsplit{j}_{i.name}", ins=[], outs=[]
                        )
                        ev.engine = i.engine
                        ev.sync_info = mybir.SyncInfo(on_wait=[wcond], on_update=[])
                        out_list.append(ev)
                    i.sync_info = mybir.SyncInfo(
                        on_wait=[si.on_wait[-1]], on_update=list(si.on_update)
                    )
                    changed = True
                out_list.append(i)
            if changed:
                bb.instructions = out_list


def _get_nc():
    global _NC
    if _NC is None:
        _NC = _build_nc()
    return _NC


# ---------------- host-side prep ----------------


def _host_mixed(v0, v1, v2, adj, wn0, wn1, wn2):
    vs = [v0[:, 0], v1[:, 0], v2[:, 0]]  # [N, 2l+1, tau]
    vmp = [np.einsum("ij,jmc->imc", adj, p) for p in vs]
    wn = [wn0, wn1, wn2]
    mixed = []
    for L in range(MAXL + 1):
        chunks = []
        for (l1, l2) in CHUNKS[L]:
            C = _CG[(l1, l2, L)]
            t = np.einsum("mnM,xmc,xne->xMce", C, vmp[l1], vmp[l2])
            chunks.append(t.reshape(N, 2 * L + 1, TAU * TAU))
        cg = np.concatenate(chunks, axis=2)
        mixed.append(np.einsum("xMk,kd->xMd", cg, wn[L]))
    return mixed  # [N, 2L+1, TAU] per L


def _host_gt(mixed):
    Gt = np.zeros((N, KTOT, PTOT), np.float32)
    for L in range(MAXL + 1):
        for ci, (l1, l2) in enumerate(CHUNKS[L]):
            C = _CG[(l1, l2, L)]  # [2l1+1, 2l2+1, 2L+1]
            blk = np.einsum("mnM,xmc->xcnM", C, mixed[l1])  # [N, 8, 2l2+1, 2L+1]
            r0 = KBASE[L] + ci * TAU
            c0 = PBASE[L] + L2BASE[l2] * (2 * L + 1)
            Gt[:, r0 : r0 + TAU, c0 : c0 + (2 * l2 + 1) * (2 * L + 1)] = blk.reshape(N, TAU, -1)
    return Gt


def _host_sel():
    SEL = np.zeros((PTOT, QTOT), np.float32)
    for L in range(MAXL + 1):
        for m2p in range(9):
            for M in range(2 * L + 1):
                SEL[PBASE[L] + m2p * (2 * L + 1) + M, QBASE[L] + M] = 1.0
    return SEL


def kernel(**inputs):
    inputs = {k: np.asarray(v, dtype=np.float32) for k, v in inputs.items()}
    v0, v1, v2 = inputs["v0"], inputs["v1"], inputs["v2"]
    adj = inputs["adj"]
    s0, s1, s2 = inputs["s0"], inputs["s1"], inputs["s2"]
    wn0, wn1, wn2 = inputs["wn0"], inputs["wn1"], inputs["wn2"]
    wr0, wr1, wr2 = inputs["wr0"], inputs["wr1"], inputs["wr2"]

    mixed = _host_mixed(v0, v1, v2, adj, wn0, wn1, wn2)
    Gt = _host_gt(mixed)  # [N, 120, 81]
    SEL = _host_sel()

    # sph, replicated across the 81 p-rows: strep_all[x, p, y] = S9[x, y, m2p(p)]
    S9 = np.concatenate([s0[:, :, :, 0], s1[:, :, :, 0], s2[:, :, :, 0]], axis=2)  # [N, N, 9]
    m2p_of_p = np.array(
        [m2p for L in range(MAXL + 1) for m2p in range(9) for _ in range(2 * L + 1)], np.int64
    )
    strep_all = np.ascontiguousarray(S9.transpose(0, 2, 1))[:, m2p_of_p, :]  # [N, 81, N]

    # wr -> wrT[x] = [c'(120), y(256)*d(8)]
    wrT = np.empty((N, KTOT, N * TAU), np.float32)
    wrT[:, 0:24] = wr0.transpose(0, 2, 1, 3).reshape(N, 24, N * TAU)
    wrT[:, 24:72] = wr1.transpose(0, 2, 1, 3).reshape(N, 48, N * TAU)
    wrT[:, 72:120] = wr2.transpose(0, 2, 1, 3).reshape(N, 48, N * TAU)

    in_maps = []
    for c in range(NCORES):
        xs = slice(c * XC, (c + 1) * XC)
        in_maps.append(
            {
                "wrt": np.ascontiguousarray(wrT[xs]),
                "gt": np.ascontiguousarray(Gt[xs].transpose(1, 0, 2)),
                "strep": np.ascontiguousarray(strep_all[xs].transpose(1, 0, 2)),
                "sel": SEL,
            }
        )

    nc = _get_nc()
    res = run_bass_kernel_spmd(nc, in_maps, core_ids=list(range(NCORES))).results

    acc = np.zeros((QTOT, N, TAU), np.float64)
    for r in res:
        acc += r["out"].reshape(QTOT, N, TAU)
    parts = [acc[0:1], acc[1:4], acc[4:9]]  # [2l+1, y, d]
    outs = [(p / p.sum()).astype(np.float32) for p in parts]
    return np.concatenate(outs, axis=0).transpose(1, 0, 2)[None]  # [1, N, 9, TAU]


# revision 23
# speedup vs baseline: 1.0017x; 1.0017x over previous
"""Trainium2 Bass kernel for nn_CGLayer (gnn_message_passing).

Strategy (x-sharded over 8 cores, 32 target-rows each):
  Host: tiny phases — vmp = adj@v, first CG product + wn mixing -> mixed[L][x,M,d];
        fold CG tables x mixed into per-x stationary matrices Gt[x] [120k, 81p];
        rearrange wr -> wrT[x] [120, (y,d)=2048] so the big c'-contraction is a
        layout-perfect PE matmul; replicate sph -> strep[x] [81, 256].
  Device per core, per x: 1MB DMA of wrT[x];
        PE mm2: V[81,(y,d)] = Gt[x].T @ wrT[x]          (k=120, f=512 x4)
        DVE:    W = V * strep[x] (broadcast over d)
        PE mm3: acc[9,(y,d)] += SEL.T @ W                (k=81, accumulate over x)
  Host: sum partials over cores, per-l global-sum normalize, concat.
"""

import math
import sys
from contextlib import ExitStack

import numpy as np

sys.path.insert(0, "/opt/trn_rl_repo")

import concourse.bass as bass
import concourse.mybir as mybir
from concourse import tile
from concourse.tile import add_dep_helper
from concourse.bass_utils import run_bass_kernel_spmd

MAXL = 2
N = 256
TAU = 8
NCORES = 8
XC = N // NCORES  # 32 x-rows per core

# ---------------- CG coefficient tables (same math as reference) ----------------


def _cg_coeff(j1, m1, j2, m2, J, M):
    if m1 + m2 != M or J < abs(j1 - j2) or J > j1 + j2:
        return 0.0
    f = math.factorial
    pre = math.sqrt(
        (2 * J + 1) * f(J + j1 - j2) * f(J - j1 + j2) * f(j1 + j2 - J) / f(j1 + j2 + J + 1)
    )
    pre *= math.sqrt(f(J + M) * f(J - M) * f(j1 - m1) * f(j1 + m1) * f(j2 - m2) * f(j2 + m2))
    kmin = max(0, j2 - J - m1, j1 + m2 - J)
    kmax = min(j1 + j2 - J, j1 - m1, j2 + m2)
    s = 0.0
    for k in range(kmin, kmax + 1):
        s += (-1.0) ** k / (
            f(k) * f(j1 + j2 - J - k) * f(j1 - m1 - k) * f(j2 + m2 - k) * f(J - j2 + m1 + k) * f(J - j1 - m2 + k)
        )
    return pre * s


def _cg_tensor(l1, l2, L):
    C = np.zeros((2 * l1 + 1, 2 * l2 + 1, 2 * L + 1), np.float32)
    for m1 in range(-l1, l1 + 1):
        for m2 in range(-l2, l2 + 1):
            M = m1 + m2
            if -L <= M <= L:
                C[m1 + l1, m2 + l2, M + L] = _cg_coeff(l1, m1, l2, m2, L, M)
    return C


_CG = {}
for _l1 in range(MAXL + 1):
    for _l2 in range(MAXL + 1):
        for _L in range(abs(_l1 - _l2), min(_l1 + _l2, MAXL) + 1):
            _CG[(_l1, _l2, _L)] = _cg_tensor(_l1, _l2, _L)

# chunk (l1,l2) lists per L, in reference concat order
CHUNKS = {
    L: [(l1, l2) for l1 in range(MAXL + 1) for l2 in range(MAXL + 1) if abs(l1 - l2) <= L <= l1 + l2]
    for L in range(MAXL + 1)
}
KBASE = {0: 0, 1: 24, 2: 72}  # k-row base per L (widths 24/48/48) -> 120 total
KTOT = 120
PBASE = {0: 0, 1: 9, 2: 36}  # p-col base per L (widths 9*(2L+1)) -> 81 total
PTOT = 81
L2BASE = [0, 1, 4]  # m2' offset of l2-block within the 9 sph rows
QBASE = [0, 1, 4]  # out row base per L -> 9 total
QTOT = 9
FB = 4  # f-blocks of 512 over (y,d)=2048

# ---------------- device kernel ----------------

_NC = None
LAST_RESULTS = None


def _build_nc():
    f32 = mybir.dt.float32
    f32r = mybir.dt.float32r
    nc = bass.Bass(trn_type="TRN2")
    wrt = nc.dram_tensor("wrt", [XC, KTOT, N * TAU], f32r, kind="ExternalInput")
    gt = nc.dram_tensor("gt", [KTOT, XC, PTOT], f32r, kind="ExternalInput")
    strep = nc.dram_tensor("strep", [PTOT, XC, N], f32, kind="ExternalInput")
    sel = nc.dram_tensor("sel", [PTOT, QTOT], f32r, kind="ExternalInput")
    out = nc.dram_tensor("out", [QTOT, N * TAU], f32, kind="ExternalOutput")

    with ExitStack() as ctx:
        tc = ctx.enter_context(tile.TileContext(nc))
        const = ctx.enter_context(tc.tile_pool(name="const", bufs=1))
        wrt_pool = ctx.enter_context(tc.tile_pool(name="wrt", bufs=6))
        w_pool = ctx.enter_context(tc.tile_pool(name="w", bufs=4))
        v_psum = ctx.enter_context(tc.tile_pool(name="v", bufs=3, space="PSUM"))
        probe_psum = ctx.enter_context(tc.tile_pool(name="probe", bufs=1, space="PSUM"))
        acc_psum = ctx.enter_context(tc.tile_pool(name="acc", bufs=1, space="PSUM"))
        outp = ctx.enter_context(tc.tile_pool(name="outp", bufs=1))
        out_sb = outp.tile([QTOT, N * TAU], f32, name="out_sb")

        gt_t = const.tile([KTOT, XC, PTOT], f32r)
        nc.sync.dma_start(gt_t[:], gt[:])
        strep_t = const.tile([PTOT, XC, N], f32)
        nc.sync.dma_start(strep_t[:], strep[:])
        sel_t = const.tile([PTOT, QTOT], f32r)
        nc.sync.dma_start(sel_t[:], sel[:])

        accs = [
            acc_psum.tile([QTOT, 64, TAU], f32, name=f"acc{fb}", tag=f"acc{fb}")
            for fb in range(FB)
        ]
        probe = probe_psum.tile([1, 1], f32, name="probe", tag="probe")

        for x in range(XC):
            wt = wrt_pool.tile([KTOT, N * TAU], f32r)
            nc.sync.dma_start(wt[:], wrt[x])
            # tiny PE matmul touching wt so PE observes the DMA semaphore here;
            # the real matmuls then carry at most one wait (fp32r LDW limit).
            probe_mm = nc.tensor.matmul(
                probe[:], wt[0:1, 0:1].bitcast(f32), wt[0:1, 0:1].bitcast(f32), start=(x == 0), stop=(x == XC - 1)
            )
            for fb in range(FB):
                v = v_psum.tile([PTOT, 64, TAU], f32)
                mm2 = nc.tensor.matmul(
                    v[:],
                    gt_t[:, x, :],
                    wt[:, fb * 512 : (fb + 1) * 512],
                    start=True,
                    stop=True,
                )
                if fb == 0:
                    add_dep_helper(mm2.ins, probe_mm.ins, sync=True, reason="mm2 after probe so PE already observed wt DMA")
                w = w_pool.tile([PTOT, 64, TAU], f32r)
                nc.vector.tensor_tensor(
                    w[:],
                    v[:],
                    strep_t[:, x, fb * 64 : (fb + 1) * 64, None].to_broadcast((PTOT, 64, TAU)),
                    mybir.AluOpType.mult,
                )
                nc.tensor.matmul(
                    accs[fb][:],
                    sel_t[:],
                    w[:],
                    start=(x == 0),
                    stop=(x == XC - 1),
                )
                if x == XC - 1:
                    # drain this fb's accumulator while later fbs still run
                    nc.vector.tensor_copy(out_sb[:, fb * 512 : (fb + 1) * 512], accs[fb][:])
                    nc.sync.dma_start(out[:, fb * 512 : (fb + 1) * 512], out_sb[:, fb * 512 : (fb + 1) * 512])
    _split_multi_waits(nc)
    return nc


# walrus' compute-op structs hold a single sync-wait slot; hoist extra waits
# into standalone single-wait EventSemaphore ops on the same engine.
_NO_SPLIT_TYPES = {"InstEventSemaphore", "InstUnconditionalBranch", "InstCall", "InstISA"}


def _split_multi_waits(nc):
    for fn in nc.m.functions:
        for bb in fn.blocks:
            insts = list(bb.instructions)
            out_list = []
            changed = False
            for i in insts:
                si = i.sync_info
                if (
                    si is not None
                    and len(si.on_wait) > 1
                    and type(i).__name__ not in _NO_SPLIT_TYPES
                ):
                    for j, wcond in enumerate(list(si.on_wait[:-1])):
                        ev = mybir.InstEventSemaphore(
                            name=f"wsplit{j}_{i.name}", ins=[], outs=[]
                        )
                        ev.engine = i.engine
                        ev.sync_info = mybir.SyncInfo(on_wait=[wcond], on_update=[])
                        out_list.append(ev)
                    i.sync_info = mybir.SyncInfo(
                        on_wait=[si.on_wait[-1]], on_update=list(si.on_update)
                    )
                    changed = True
                out_list.append(i)
            if changed:
                bb.instructions = out_list


def _get_nc():
    global _NC
    if _NC is None:
        _NC = _build_nc()
    return _NC


# ---------------- host-side prep ----------------


def _host_mixed(v0, v1, v2, adj, wn0, wn1, wn2):
    vs = [v0[:, 0], v1[:, 0], v2[:, 0]]  # [N, 2l+1, tau]
    vmp = [np.einsum("ij,jmc->imc", adj, p) for p in vs]
    wn = [wn0, wn1, wn2]
    mixed = []
    for L in range(MAXL + 1):
        chunks = []
        for (l1, l2) in CHUNKS[L]:
            C = _CG[(l1, l2, L)]
            t = np.einsum("mnM,xmc,xne->xMce", C, vmp[l1], vmp[l2])
            chunks.append(t.reshape(N, 2 * L + 1, TAU * TAU))
        cg = np.concatenate(chunks, axis=2)
        mixed.append(np.einsum("xMk,kd->xMd", cg, wn[L]))
    return mixed  # [N, 2L+1, TAU] per L


def _host_gt(mixed):
    Gt = np.zeros((N, KTOT, PTOT), np.float32)
    for L in range(MAXL + 1):
        for ci, (l1, l2) in enumerate(CHUNKS[L]):
            C = _CG[(l1, l2, L)]  # [2l1+1, 2l2+1, 2L+1]
            blk = np.einsum("mnM,xmc->xcnM", C, mixed[l1])  # [N, 8, 2l2+1, 2L+1]
            r0 = KBASE[L] + ci * TAU
            c0 = PBASE[L] + L2BASE[l2] * (2 * L + 1)
            Gt[:, r0 : r0 + TAU, c0 : c0 + (2 * l2 + 1) * (2 * L + 1)] = blk.reshape(N, TAU, -1)
    return Gt


def _host_sel():
    SEL = np.zeros((PTOT, QTOT), np.float32)
    for L in range(MAXL + 1):
        for m2p in range(9):
            for M in range(2 * L + 1):
                SEL[PBASE[L] + m2p * (2 * L + 1) + M, QBASE[L] + M] = 1.0
    return SEL


def kernel(**inputs):
    inputs = {k: np.asarray(v, dtype=np.float32) for k, v in inputs.items()}
    v0, v1, v2 = inputs["v0"], inputs["v1"], inputs["v2"]
    adj = inputs["adj"]
    s0, s1, s2 = inputs["s0"], inputs["s1"], inputs["s2"]
    wn0, wn1, wn2 = inputs["wn0"], inputs["wn1"], inputs["wn2"]
    wr0, wr1, wr2 = inputs["wr0"], inputs["wr1"], inputs["wr2"]

    mixed = _host_mixed(v0, v1, v2, adj, wn0, wn1, wn2)
    Gt = _host_gt(mixed)  # [N, 120, 81]
    SEL = _host_sel()

    # sph, replicated across the 81 p-rows: strep_all[x, p, y] = S9[x, y, m2p(p)]
    S9 = np.concatenate([s0[:, :, :, 0], s1[:, :, :, 0], s2[:, :, :, 0]], axis=2)  # [N, N, 9]
    m2p_of_p = np.array(
        [m2p for L in range(MAXL + 1) for m2p in range(9) for _ in range(2 * L + 1)], np.int64
    )
    strep_all = np.ascontiguousarray(S9.transpose(0, 2, 1))[:, m2p_of_p, :]  # [N, 81, N]

    # wr -> wrT[x] = [c'(120), y(256)*d(8)]
    wrT = np.empty((N, KTOT, N * TAU), np.float32)
    wrT[:, 0:24] = wr0.transpose(0, 2, 1, 3).reshape(N, 24, N * TAU)
    wrT[:, 24:72] = wr1.transpose(0, 2, 1, 3).reshape(N, 48, N * TAU)
    wrT[:, 72:120] = wr2.transpose(0, 2, 1, 3).reshape(N, 48, N * TAU)

    in_maps = []
    for c in range(NCORES):
        xs = slice(c * XC, (c + 1) * XC)
        in_maps.append(
            {
                "wrt": np.ascontiguousarray(wrT[xs]),
                "gt": np.ascontiguousarray(Gt[xs].transpose(1, 0, 2)),
                "strep": np.ascontiguousarray(strep_all[xs].transpose(1, 0, 2)),
                "sel": SEL,
            }
        )

    nc = _get_nc()
    res = run_bass_kernel_spmd(nc, in_maps, core_ids=list(range(NCORES))).results

    acc = np.zeros((QTOT, N, TAU), np.float64)
    for r in res:
        acc += r["out"].reshape(QTOT, N, TAU)
    parts = [acc[0:1], acc[1:4], acc[4:9]]  # [2l+1, y, d]
    outs = [(p / p.sum()).astype(np.float32) for p in parts]
    return np.concatenate(outs, axis=0).transpose(1, 0, 2)[None]  # [1, N, 9, TAU]
